# revision 2
# baseline (speedup 1.0000x reference)
"""Multi-head causal attention (B=2, T=2048, H=1024, 16 heads) on 8 Trainium2
NeuronCores.

Sharding: data-parallel over batch (2 groups of 4 cores) x tensor-parallel over
heads (4 heads/core). Each core computes the qkv projection for its heads,
rotary embedding, causal+padding-masked attention, and its partial
out-projection; a ReduceScatter over each 4-core group combines the out-proj
partials and the host concatenates the shards.

Structure (per core, per rep):
- Phase A: qk^T projection into pair-packed [128, T] tiles (PSUM 4-ring),
  bias via ScalarE-free per-partition add, rotate_half as a PE permutation
  matmul (no DMA), V projection trimmed to its live 260 columns with the
  padding mask and bias fused (ones columns make every PV matmul also
  produce softmax rowsums).
- Phase B: scores computed transposed (keys on partitions) one 128-key chunk
  at a time into a [128, 2, 512] PSUM 2-ring; exp on ScalarE paces the loop
  while TensorE runs a software pipeline (scores of chunk c issue before the
  PV matmuls of chunk c-1, continuing across head-pair boundaries). Causal
  masking multiplies 0/1 strips post-exp; diagonal chunks skip the fully
  masked query columns entirely. Softmax normalization (reciprocal +
  broadcast matmul + multiply) and the out-projection are deferred and
  drained one item per chunk into the same stream, so no engine idles at
  group boundaries.
"""
import sys

for _p in ("/opt/trn_rl_repo", "/root/.axon_site/_ro/trn_rl_repo"):
    if _p not in sys.path:
        sys.path.append(_p)

from contextlib import ExitStack

import numpy as np
import ml_dtypes

import concourse.bacc as bacc
import concourse.tile as tile
from concourse import mybir
from concourse.bass_utils import run_bass_kernel_spmd

BF16 = ml_dtypes.bfloat16
F32 = mybir.dt.float32
BF = mybir.dt.bfloat16

N_CORES = 8
B, T, H = 2, 2048, 1024
NH, HD = 16, 64
HPC = 4
NKC = T // 128
NQT = T // 512
VW = 4 * 65
ROPE_BASE = 10000.0

_PROGRAMS = {}


def _emit_body(ctx, tc, io, pools, phases=(1, 2, 3, 4)):
    nc = tc.nc
    mult = mybir.AluOpType.mult
    add = mybir.AluOpType.add
    AF = mybir.ActivationFunctionType

    (xT_sb, wqk_sb, bqk_sb, wv_sb, bv_sb, wr_sb, cos_sb, sin_sb, caus_sb,
     km_sb, ones_bf, ones_f32, perm_sb, v_sb) = pools["consts"]
    kz = pools["kz"]          # 4 persistent [128,T] tiles, zero halves preset
    p_qkraw = pools["qkraw"]
    p_rope = pools["rope"]
    p_qf = pools["qf"]
    p_on = pools["onorm"]
    p_E = pools["E"]
    p_sm = pools["small"]
    p_ysb = pools["ysb"]
    y_int = io["y_int"]

    # ---- phase A: projections ------------------------------------------
    qf = []
    with tc.tile_pool(name="psA", bufs=1, space="PSUM") as psA:
        # qk^T projection (pair-packed rows) + bias + rope.
        # M-chunks: 0,1 = q pairs (heads 01, 23); 2,3 = k pairs.
        for m in range(4 if 1 in phases else 0):
            qkraw = p_qkraw.tile([128, T], BF, tag="qkraw")
            if m < 2:
                qfm = p_qf.tile([128, T], BF, tag="qf", name=f"qf{m}")
            for nth in range(NQT):
                ts = slice(512 * nth, 512 * nth + 512)
                ps = psA.tile([128, 512], F32, tag="qk", bufs=4,
                              name="psqk")
                for k in range(8):
                    nc.tensor.matmul(
                        ps[:],
                        wqk_sb[:, k, 128 * m:128 * m + 128],
                        xT_sb[:, k, ts],
                        start=(k == 0), stop=(k == 7),
                    )
                nc.vector.tensor_scalar(qkraw[:, ts], ps[:],
                                        bqk_sb[:, m:m + 1], None, add)
                # rotate_half: partition permutation on PE (sign folded
                # into the sin table)
                shp = psA.tile([128, 512], F32, tag="qk", bufs=4,
                               name="shp")
                nc.tensor.matmul(shp[:], perm_sb[:], qkraw[:, ts],
                                 start=True, stop=True)
                t1 = p_rope.tile([128, 512], BF, tag="tmp")
                nc.vector.tensor_tensor(t1[:], qkraw[:, ts],
                                        cos_sb[:, ts], mult)
                t2 = p_rope.tile([128, 512], BF, tag="tmp")
                nc.vector.tensor_tensor(t2[:], shp[:], sin_sb[:, ts], mult)
                if m < 2:
                    nc.vector.tensor_tensor(qfm[:, ts], t1[:], t2[:], add)
                else:
                    for e in range(2):
                        lo, hi = 64 * e, 64 * e + 64
                        nc.vector.tensor_tensor(
                            kz[2 * (m - 2) + e][lo:hi, ts],
                            t1[lo:hi, :], t2[lo:hi, :], add)
            if m < 2:
                qf.append(qfm)

        # v projection (+ones columns, +bias); the padding mask multiply is
        # fused into the PSUM evacuation.
        for c in range(NKC if 2 in phases else 0):
            psv = psA.tile([128, VW], F32, tag="v", bufs=4, name="psv")
            nc.tensor.matmul(psv[:], ones_bf[0:1, :], bv_sb[:],
                             start=True, stop=False)
            for k in range(8):
                nc.tensor.matmul(
                    psv[:],
                    xT_sb[:, k, 128 * c:128 * c + 128],
                    wv_sb[:, k, :],
                    start=False, stop=(k == 7),
                )
            nc.vector.tensor_scalar(v_sb[:, c, :], psv[:],
                                    km_sb[:, c:c + 1], None, mult)

    # ---- phase B: attention, with deferred normalize/out-proj ----------
    onp = [p_on.tile([128, T], BF, tag="onp", name=f"onp{p}")
           for p in range(2)]
    with tc.tile_pool(name="psB", bufs=1, space="PSUM") as psB:
        deferred = []  # queue of zero-arg emit callbacks

        def emit_norm(nt, p, Oacc, e, shared):
            # e==0 drain computes the shared reciprocal for both heads
            def fn():
                if e == 0:
                    rcp = p_sm.tile([65, 1024], F32, tag="rcp", name="rcp")
                    nc.vector.reciprocal(
                        rcp[64:65, :],
                        Oacc[64:65, :, :].rearrange("p a b -> p (a b)"))
                    shared["rcp"] = rcp
                rcp = shared["rcp"]
                bcp = psB.tile([64, 512], F32, tag="big", bufs=2,
                               name="bcp")
                nc.tensor.matmul(bcp[:], ones_f32[64:65, 0:64],
                                 rcp[64:65, 512 * e:512 * e + 512],
                                 start=True, stop=True)
                bcs = p_sm.tile([64, 512], F32, tag="bcs", name="bcs")
                nc.vector.tensor_copy(bcs[:], bcp[:])
                hs = slice(512 * nt, 512 * nt + 512)
                if e == 0:
                    nc.vector.tensor_tensor(onp[p][0:64, hs],
                                            Oacc[0:64, e, :], bcs[:], mult)
                else:
                    ot = p_ysb.tile([64, 512], BF, tag="ot", name="ot")
                    nc.vector.tensor_tensor(ot[:], Oacc[0:64, e, :],
                                            bcs[:], mult)
                    nc.sync.dma_start(onp[p][64:128, hs], ot[:])
            return fn

        def emit_outproj(qs):
            def fn():
                py = psB.tile([128, 2, 512], F32, tag="big", bufs=2,
                              name="py")
                for ns in range(2):
                    for p in range(2):
                        nc.tensor.matmul(
                            py[:, ns, :],
                            onp[p][:, 128 * qs:128 * qs + 128],
                            wr_sb[:, p, 512 * ns:512 * ns + 512],
                            start=(p == 0), stop=(p == 1),
                        )
                ysb = p_ysb.tile([128, 1024], BF, tag="ysb")
                nc.vector.tensor_copy(
                    ysb[:].rearrange("p (a b) -> p a b", a=2), py[:])
                nc.sync.dma_start(y_int[128 * qs:128 * qs + 128, :],
                                  ysb[:])
            return fn

        # continuous software pipeline across all (nt, pair) groups: the PV
        # matmuls lag the score matmuls by one chunk so TensorE never parks
        # on a PV waiting for its exp, even across group boundaries.
        pend = [None]  # (emit_pv, c, E, on_last)

        def flush_pend():
            if pend[0] is not None:
                emit_pv, c, E, on_last = pend[0]
                emit_pv(c, E)
                if on_last is not None:
                    on_last()
                pend[0] = None

        for nt in range(NQT if 3 in phases else 0):
            nch = 4 * nt + 4
            for p in range(2):
                qT = qf[p]
                Oacc = psB.tile([65, 2, 512], F32, tag="Oacc", bufs=2,
                                name="Oacc")

                def emit_pv(cq, E, nch=nch, Oacc=Oacc, p=p):
                    c, q0 = cq
                    for e in range(2):
                        lh = 2 * p + e
                        nc.tensor.matmul(
                            Oacc[:, e, q0:],
                            v_sb[:, c, 65 * lh:65 * lh + 65],
                            E[:, e, q0:],
                            start=(c == 0), stop=(c == nch - 1),
                        )

                def on_pair_done(nt=nt, p=p, Oacc=Oacc):
                    shared = {}
                    for e in range(2):
                        deferred.append(emit_norm(nt, p, Oacc, e, shared))
                    if p == 1 and 4 in phases:
                        for qs in range(4 * nt, 4 * nt + 4):
                            deferred.append(emit_outproj(qs))

                for c in range(nch):
                    # diagonal chunks: queries below the causal boundary
                    # are fully masked — skip their columns entirely
                    off = c - 4 * nt
                    q0 = 128 * off if off > 0 else 0
                    Sp = psB.tile([128, 2, 512], F32, tag="big", bufs=2,
                                  name="Sp")
                    for e in range(2):
                        nc.tensor.matmul(
                            Sp[:, e, q0:],
                            kz[2 * p + e][:, 128 * c:128 * c + 128],
                            qT[:, 512 * nt + q0:512 * nt + 512],
                            start=True, stop=True,
                        )
                    E = p_E.tile([128, 2, 512], BF, tag="E")
                    nc.scalar.activation(E[:, :, q0:], Sp[:, :, q0:],
                                         AF.Exp, scale=0.125)
                    if off >= 0:
                        nc.vector.tensor_tensor(
                            E[:, :, q0:], E[:, :, q0:],
                            caus_sb[:, 1024 * off:1024 * off + 1024]
                            .rearrange("p (a b) -> p a b", a=2)[:, :, q0:],
                            mult)
                    flush_pend()
                    pend[0] = (emit_pv, (c, q0), E,
                               on_pair_done if c == nch - 1 else None)
                    # drain one deferred item per chunk (earlier groups
                    # whose inputs are long ready)
                    if deferred and c >= 1:
                        deferred.pop(0)()
        flush_pend()
        for fn in deferred:
            fn()


def build_program(nreps=1, use_collective=True, phases=(1, 2, 3, 4)):
    key = (nreps, use_collective, tuple(phases))
    if key in _PROGRAMS:
        return _PROGRAMS[key]

    nc = bacc.Bacc("TRN2", target_bir_lowering=False, debug=False,
                   num_devices=N_CORES)
    xT = nc.dram_tensor("xT", [H, T], BF, kind="ExternalInput")
    wqk = nc.dram_tensor("wqk", [H, 512], BF, kind="ExternalInput")
    bqkT = nc.dram_tensor("bqkT", [128, 4], F32, kind="ExternalInput")
    wv = nc.dram_tensor("wv", [H, VW], BF, kind="ExternalInput")
    bv = nc.dram_tensor("bv", [1, VW], BF, kind="ExternalInput")
    wr = nc.dram_tensor("wr", [2, 128, H], BF, kind="ExternalInput")
    cosT = nc.dram_tensor("cosT", [128, T], BF, kind="ExternalInput")
    sinT = nc.dram_tensor("sinT", [128, T], BF, kind="ExternalInput")
    caus = nc.dram_tensor("caus", [128, 4096], BF, kind="ExternalInput")
    kmT = nc.dram_tensor("kmT", [128, NKC], F32, kind="ExternalInput")
    perm = nc.dram_tensor("perm", [128, 128], BF, kind="ExternalInput")
    out_shape = [T // 4, H] if use_collective else [T, H]
    yout = nc.dram_tensor("y", out_shape, BF, kind="ExternalOutput")

    with tile.TileContext(nc) as tc, ExitStack() as ctx:
        const = ctx.enter_context(tc.tile_pool(name="const", bufs=1))
        p_qkraw = ctx.enter_context(tc.tile_pool(name="qkraw", bufs=2))
        p_rope = ctx.enter_context(tc.tile_pool(name="rope", bufs=4))
        p_qf = ctx.enter_context(tc.tile_pool(name="qf", bufs=2))
        p_on = ctx.enter_context(tc.tile_pool(name="onorm", bufs=2))
        p_E = ctx.enter_context(tc.tile_pool(name="E", bufs=4))
        p_sm = ctx.enter_context(tc.tile_pool(name="small", bufs=3))
        p_ysb = ctx.enter_context(tc.tile_pool(name="ysb", bufs=3))
        dram = ctx.enter_context(tc.tile_pool(name="dram", bufs=1,
                                              space="DRAM"))

        xT_sb = const.tile([128, 8, T], BF)
        nc.sync.dma_start(xT_sb[:], xT.ap().rearrange("(k p) t -> p k t",
                                                      p=128))
        wqk_sb = const.tile([128, 8, 512], BF)
        nc.sync.dma_start(wqk_sb[:], wqk.ap().rearrange("(k p) m -> p k m",
                                                        p=128))
        bqk_sb = const.tile([128, 4], F32)
        nc.sync.dma_start(bqk_sb[:], bqkT.ap())
        wv_sb = const.tile([128, 8, VW], BF)
        nc.sync.dma_start(wv_sb[:], wv.ap().rearrange("(k p) m -> p k m",
                                                      p=128))
        bv_sb = const.tile([1, VW], BF)
        nc.sync.dma_start(bv_sb[:], bv.ap())
        wr_sb = const.tile([128, 2, H], BF)
        nc.sync.dma_start(wr_sb[:], wr.ap().rearrange("h p m -> p h m"))
        cos_sb = const.tile([128, T], BF)
        nc.sync.dma_start(cos_sb[:], cosT.ap())
        sin_sb = const.tile([128, T], BF)
        nc.sync.dma_start(sin_sb[:], sinT.ap())
        caus_sb = const.tile([128, 4096], BF)
        nc.sync.dma_start(caus_sb[:], caus.ap())
        km_sb = const.tile([128, NKC], F32)
        nc.sync.dma_start(km_sb[:], kmT.ap())
        perm_sb = const.tile([128, 128], BF)
        nc.sync.dma_start(perm_sb[:], perm.ap())
        ones_bf = const.tile([1, 128], BF)
        nc.vector.memset(ones_bf[:], 1.0)
        ones_f32 = const.tile([128, 64], F32)
        nc.vector.memset(ones_f32[:], 1.0)
        v_sb = const.tile([128, NKC, VW], BF)

        # persistent kz tiles: zero halves set once, live halves rewritten
        # by every rep
        kz = []
        for pair in range(2):
            for e in range(2):
                kze = const.tile([128, T], BF, name=f"kz{pair}_{e}")
                nc.vector.memset(kze[64 - 64 * e:128 - 64 * e, :], 0.0)
                kz.append(kze)

        y_int = dram.tile([T, H], BF, tag="yint")

        pools = dict(
            consts=(xT_sb, wqk_sb, bqk_sb, wv_sb, bv_sb, wr_sb, cos_sb,
                    sin_sb, caus_sb, km_sb, ones_bf, ones_f32, perm_sb,
                    v_sb),
            kz=kz,
            qkraw=p_qkraw, rope=p_rope, qf=p_qf, onorm=p_on, E=p_E,
            small=p_sm, ysb=p_ysb,
        )
        io = dict(y_int=y_int)

        for _ in range(nreps):
            _emit_body(ctx, tc, io, pools, phases=phases)

            if use_collective:
                rs_out = dram.tile([T // 4, H], BF, tag="rs")
                nc.gpsimd.collective_compute(
                    "ReduceScatter", mybir.AluOpType.add,
                    replica_groups=[[0, 1, 2, 3], [4, 5, 6, 7]],
                    ins=[y_int.opt()], outs=[rs_out.opt()],
                )
                nc.gpsimd.dma_start(yout.ap(), rs_out[:])
            else:
                nc.sync.dma_start(yout.ap(), y_int[:])

    nc.compile()
    _PROGRAMS[key] = nc
    return nc


def make_inputs(hidden_state, attention_mask, w_qkv, b_qkv, w_out):
    """Host-side shard prep. Returns one input dict per core."""
    hidden_state = np.asarray(hidden_state)
    attention_mask = np.asarray(attention_mask)
    w_qkv = np.asarray(w_qkv)
    b_qkv = np.asarray(b_qkv)
    w_out = np.asarray(w_out)

    inv_freq = 1.0 / (ROPE_BASE ** (np.arange(0, HD, 2, dtype=np.float32)
                                    / HD))
    t = np.arange(T, dtype=np.float32)
    freqs = np.outer(t, inv_freq)
    emb = np.concatenate([freqs, freqs], axis=-1)
    cosT = np.cos(emb).T.astype(np.float32)
    sinT = np.sin(emb).T.astype(np.float32)
    sin_eff = sinT.copy()
    sin_eff[:32] = -sin_eff[:32]
    cos_pair = np.vstack([cosT, cosT]).astype(BF16)
    sin_pair = np.vstack([sin_eff, sin_eff]).astype(BF16)

    # rotate_half partition permutation: sh[dst] = x[src] for
    # (0,32),(32,0),(64,96),(96,64) blocks of 32; perm[k, i] = 1 iff
    # k == sigma(i) (lhsT layout: out[i] = sum_k perm[k, i] x[k])
    perm = np.zeros((128, 128), dtype=BF16)
    for dst, src in ((0, 32), (32, 0), (64, 96), (96, 64)):
        for j in range(32):
            perm[src + j, dst + j] = 1.0

    dk = np.arange(128)[:, None]
    dq = np.arange(512)[None, :]
    caus = np.zeros((128, 4096), dtype=BF16)
    for off in range(4):
        pat = (dq >= dk + 128 * off).astype(BF16)
        caus[:, 1024 * off:1024 * off + 512] = pat
        caus[:, 1024 * off + 512:1024 * off + 1024] = pat

    in_maps = []
    for core in range(N_CORES):
        b = core // 4
        hg = core % 4
        heads = [4 * hg + j for j in range(HPC)]

        cols_q = np.concatenate([np.arange(h * 192, h * 192 + 64)
                                 for h in heads])
        cols_k = cols_q + 64
        cols_v = cols_q + 128
        wqk = w_qkv[:, np.concatenate([cols_q, cols_k])].astype(BF16)
        bqk = b_qkv[np.concatenate([cols_q, cols_k])].astype(np.float32)
        bqkT = bqk.reshape(4, 128).T.copy()

        wv = np.zeros((H, VW), dtype=BF16)
        bv = np.zeros((1, VW), dtype=BF16)
        for j, h in enumerate(heads):
            wv[:, 65 * j:65 * j + 64] = w_qkv[:, cols_v[64 * j:64 * j + 64]]
            bv[0, 65 * j:65 * j + 64] = b_qkv[cols_v[64 * j:64 * j + 64]]
            bv[0, 65 * j + 64] = 1.0

        wr = w_out[256 * hg:256 * hg + 256, :].reshape(2, 128, H) \
            .astype(BF16)

        kmT = (attention_mask[b].reshape(NKC, 128).T != 0) \
            .astype(np.float32)

        in_maps.append({
            "xT": np.ascontiguousarray(hidden_state[b].T).astype(BF16),
            "wqk": np.ascontiguousarray(wqk),
            "bqkT": bqkT,
            "wv": wv,
            "bv": bv,
            "wr": wr,
            "cosT": cos_pair,
            "sinT": sin_pair,
            "caus": caus,
            "kmT": kmT,
            "perm": perm,
        })
    return in_maps


def kernel(hidden_state, attention_mask, w_qkv, b_qkv, w_out,
           _use_collective=True):
    nc = build_program(nreps=1, use_collective=_use_collective)
    in_maps = make_inputs(hidden_state, attention_mask, w_qkv, b_qkv, w_out)
    res = run_bass_kernel_spmd(nc, in_maps, list(range(N_CORES))).results

    out = np.empty((B, T, H), dtype=np.float32)
    if _use_collective:
        for core in range(N_CORES):
            b, j = core // 4, core % 4
            out[b, 512 * j:512 * (j + 1), :] = \
                res[core]["y"].astype(np.float32)
    else:
        for b in range(B):
            out[b] = sum(res[4 * b + j]["y"].astype(np.float32)
                         for j in range(4))
    return out
